# revision 22
# baseline (speedup 1.0000x reference)
"""Gumbel-Sinkhorn (masked, 5 iterations) on Trainium2, data-parallel over 8 cores.

Math: the reference's masked log-domain Sinkhorn equals, in the probability
domain, classic Sinkhorn scaling of K = exp(masked_logits):

    v_0 = 1;  u_k = 1/(K v_{k-1} + eps);  v_k = 1/(K^T u_k + eps)   (k = 1..5)
    out = K * (u_5 outer v_5) * exp(1e-6),   masked entries exactly 0.

V2: fp16 HBM I/O (halves DMA, the roofline term), float32r compute on-chip.
K^T is built with PE transposes instead of loading a host-transposed copy
(v1 spent 16 MiB/core of HBM traffic on that). The eps guard rides on the PE
as a rank-1 PSUM accumulate, so no DVE max pass. Final out = K*(u x v):
PE broadcasts v-rows per sample into PSUM, ACT/DVE apply the per-partition
u scale (w = ps_b * u, written into the dead et tile), DVE/Pool multiply.

Emission is software-pipelined over cohort PAIRS so the per-engine in-order
streams interleave across cohorts: while pair p iterates (PE/DVE ping-pong),
ACT runs pair p+1's exp and the final-phase scale-copies of pair p-1, Pool
multiplies pair p-1, DMA streams pair p+1 in / p-1 out. Within a pair the
ten matvec-reciprocal phases alternate cohorts A/B so each engine's
dependency stalls are filled by the sibling cohort.

Host-side prep (cheap numpy): mask logits to -1e4, cast fp16; output is cast
back to fp32. All O(B*A*T) math runs on device.
"""

import numpy as np

B, A, T = 512, 256, 256
NCORES = 8
BPC = B // NCORES          # samples per core
C = 8                      # cohort size (samples in lockstep)
G = BPC // C               # cohorts per core
P = G // 2                 # cohort pairs (pipeline unit)
ITERS = 5
MASKVAL = np.float16(-1e4)  # exp(-1e4) == 0.0 exactly
EPS = 1e-18                 # rank-1 PSUM bias; guards 1/0 on masked rows/cols.
                            # Added (not max'ed) into every row/col sum, so it must
                            # sit far below the smallest valid sum (~1e-13) while
                            # keeping 1/EPS^2 = 1e36 finite in fp32 for fully-masked
                            # row x column pairs in the u*v broadcast.
OUT_SCALE = float(np.exp(np.float64(1e-6)))  # reference's exp(x + 1e-6)

# --- engine-assignment knobs (per cohort) ---
ET_ACT = 5     # of the 8 et-drain chunks per cohort, how many go to ACT
W_ACT = 5      # of the 16 w scale-copies per cohort, how many go to ACT
MUL_POOL = 3   # of the 4 [128,1024] mul chunks per cohort on Pool (rest DVE)
TR_WEAVE_FROM = 4  # first iteration phase that carries transpose units

_NC_CACHE = None


def _build_nc():
    import concourse.tile as tile
    from concourse import bacc, mybir

    f32 = mybir.dt.float32
    f32r = mybir.dt.float32r
    fp16 = mybir.dt.float16
    AF = mybir.ActivationFunctionType

    nc = bacc.Bacc()
    lg = nc.dram_tensor("lg", [BPC, A, T], fp16, kind="ExternalInput")
    ident = nc.dram_tensor("ident", [128, 128], f32r, kind="ExternalInput")
    identsc = nc.dram_tensor("identsc", [128, 128], f32r, kind="ExternalInput")
    # sel[k, b*128+m] = 1 if k == b else 0 (v-row -> per-sample broadcast)
    sel = nc.dram_tensor("sel", [C, C * 128], f32r, kind="ExternalInput")
    # consts[0, 0:128] = EPS; consts[0, 128:128+4C] = 1.0
    consts = nc.dram_tensor("consts", [1, 256], f32r, kind="ExternalInput")
    onesd = nc.dram_tensor("onesd", [128, 4 * C], f32r, kind="ExternalInput")
    out = nc.dram_tensor("out", [BPC, A, T], fp16, kind="ExternalOutput")

    SLAB = C * 512  # free elems per cohort slab (per sample: 2 halves x 256)

    with tile.TileContext(nc) as tc:
        with (
            tc.tile_pool(name="itp", bufs=5) as itp,
            tc.tile_pool(name="e0p", bufs=4) as e0p,
            tc.tile_pool(name="etp", bufs=4) as etp,
            tc.tile_pool(name="uvp", bufs=10) as uvp,
            tc.tile_pool(name="vrowp", bufs=2) as vrowp,
            tc.tile_pool(name="constp", bufs=1) as constp,
            tc.tile_pool(name="pstp", bufs=3, space="PSUM") as pstp,
            tc.tile_pool(name="psuv", bufs=2, space="PSUM") as psuv,
            tc.tile_pool(name="psb", bufs=3, space="PSUM") as psbp,
        ):
            idr = constp.tile([128, 128], f32r)
            nc.sync.dma_start(idr[:], ident[:])
            idsc = constp.tile([128, 128], f32r)
            nc.sync.dma_start(idsc[:], identsc[:])
            sel_sb = constp.tile([C, C * 128], f32r)
            nc.sync.dma_start(sel_sb[0:C, :], sel[:])
            cst = constp.tile([1, 256], f32r)
            nc.sync.dma_start(cst[:], consts[:])
            ones2 = constp.tile([128, 4 * C], f32r)
            nc.sync.dma_start(ones2[:], onesd[:])

            st = {}  # per-cohort pipeline state: tiles

            def emit_ld(g):
                it = itp.tile([128, SLAB], fp16, name="it")
                hc = C // 2
                for h in range(2):
                    src = lg[g * C + h * hc:g * C + (h + 1) * hc].rearrange(
                        "b (h p) j -> p b h j", p=128)
                    nc.sync.dma_start(
                        it[:, h * hc * 512:(h + 1) * hc * 512].rearrange(
                            "p (b h j) -> p b h j", h=2, j=256), src)
                st[g] = {"it": it}

            def emit_ex(g):
                it = st[g]["it"]
                e0r = e0p.tile([128, SLAB], f32r, name="e0r")
                for h in range(4):
                    sl = slice(h * SLAB // 4, (h + 1) * SLAB // 4)
                    nc.scalar.activation(e0r[:, sl], it[:, sl], AF.Exp)
                st[g]["e0r"] = e0r

            def emit_tr_unit(g, b):
                # et[p=j', b*512+jt*256+ia*128+i'] = K[b, ia*128+i', jt*128+j']
                e0r, et = st[g]["e0r"], st[g]["et"]
                pst = pstp.tile([128, 512], f32r, name="pst")
                for ia in range(2):
                    for jt in range(2):
                        nc.tensor.transpose(
                            pst[:, jt * 256 + ia * 128:
                                jt * 256 + ia * 128 + 128],
                            e0r[:, b * 512 + ia * 256 + jt * 128:
                                b * 512 + ia * 256 + jt * 128 + 128],
                            idr[:],
                        )
                dst = et[:, b * 512:(b + 1) * 512]
                if b < ET_ACT:
                    nc.scalar.activation(dst, pst[:], AF.Copy)
                else:
                    nc.vector.tensor_copy(dst, pst[:])

            def emit_tr_alloc(g):
                st[g]["et"] = etp.tile([128, SLAB], f32r, name="et")

            def _phase_u(g):
                et, v2 = st[g]["et"], st[g]["v2"]
                ps_u = psuv.tile([128, 4 * C], f32, name="ps_uv")
                nc.tensor.matmul(ps_u[:], lhsT=cst[:, 0:128],
                                 rhs=cst[:, 128:128 + 4 * C],
                                 start=True, stop=False, skip_group_check=True)
                for b in range(C):
                    for ia in range(2):
                        col = 2 * (ia * C + b)
                        for jt in range(2):
                            nc.tensor.matmul(
                                ps_u[:, col:col + 2],
                                lhsT=et[:, b * 512 + jt * 256 + ia * 128:
                                        b * 512 + jt * 256 + ia * 128 + 128],
                                rhs=v2[:, 2 * (jt * C + b):2 * (jt * C + b) + 2],
                                start=False, stop=(jt == 1),
                                skip_group_check=True,
                            )
                u2 = uvp.tile([128, 4 * C], f32r, name="u2")
                with nc.allow_low_precision(reason="f32r scale vectors"):
                    nc.vector.reciprocal(
                        u2[:].rearrange("p (c d) -> p c d", d=2),
                        ps_u[:].rearrange("p (c d) -> p c d", d=2)
                        [:, :, 0:1].broadcast_to([128, 2 * C, 2]))
                st[g]["u2"] = u2

            def _phase_v(g):
                e0r, u2 = st[g]["e0r"], st[g]["u2"]
                ps_v = psuv.tile([128, 4 * C], f32, name="ps_uv")
                nc.tensor.matmul(ps_v[:], lhsT=cst[:, 0:128],
                                 rhs=cst[:, 128:128 + 4 * C],
                                 start=True, stop=False, skip_group_check=True)
                for b in range(C):
                    for jt in range(2):
                        col = 2 * (jt * C + b)
                        for ia in range(2):
                            nc.tensor.matmul(
                                ps_v[:, col:col + 2],
                                lhsT=e0r[:, b * 512 + ia * 256 + jt * 128:
                                         b * 512 + ia * 256 + jt * 128 + 128],
                                rhs=u2[:, 2 * (ia * C + b):2 * (ia * C + b) + 2],
                                start=False, stop=(ia == 1),
                                skip_group_check=True,
                            )
                v2 = uvp.tile([128, 4 * C], f32r, name="v2")
                with nc.allow_low_precision(reason="f32r scale vectors"):
                    nc.vector.reciprocal(
                        v2[:].rearrange("p (c d) -> p c d", d=2),
                        ps_v[:].rearrange("p (c d) -> p c d", d=2)
                        [:, :, 0:1].broadcast_to([128, 2 * C, 2]))
                st[g]["v2"] = v2

            def it_pair_gen(a, b):
                # yields once per phase-half (10 yields for 5 iterations);
                # the scheduler weaves other work between phase-halves
                for g in (a, b):
                    st[g]["v2"] = ones2
                for itn in range(ITERS):
                    for half in range(2):
                        for g in (a, b):
                            if half == 0:
                                _phase_u(g)
                            else:
                                _phase_v(g)
                        yield

            def _fn_pre(g):
                u2, v2 = st[g]["u2"], st[g]["v2"]
                u32 = uvp.tile([128, 2 * C], f32, name="u32")
                nc.vector.tensor_copy(u32[:], u2[:, 0:4 * C:2])
                ps_vr = psbp.tile([C, 256], f32r, name="ps_b")
                for jt in range(2):
                    nc.tensor.transpose(
                        ps_vr[0:C, jt * 128:(jt + 1) * 128],
                        v2[:, 2 * jt * C:2 * (jt + 1) * C:2],
                        idsc[:],
                    )
                vrow = vrowp.tile([C, 256], f32r, name="vrow")
                nc.vector.tensor_copy(vrow[0:C, :], ps_vr[0:C, :])
                st[g]["u32"], st[g]["vrow"] = u32, vrow

            def _fn_bcast2(g, b2):
                # one psum bank holds two samples' v-row broadcasts
                vrow = st[g]["vrow"]
                ps_b = psbp.tile([128, 512], f32, name="ps_b")
                for o in range(2):
                    b = 2 * b2 + o
                    nc.tensor.matmul(
                        ps_b[:, o * 256:(o + 1) * 256],
                        lhsT=sel_sb[0:C, b * 128:(b + 1) * 128],
                        rhs=vrow[0:C, :], start=True, stop=True,
                    )
                st[g].setdefault("ps_b", {})[b2] = ps_b

            def _fn_sample(g, b, wi):
                # w = ps_b * u for sample b, written into the dead et tile
                et, u32 = st[g]["et"], st[g]["u32"]
                ps_b = st[g]["ps_b"][b // 2]
                o = (b % 2) * 256
                for ia in range(2):
                    wsl = slice(b * 512 + ia * 256, b * 512 + (ia + 1) * 256)
                    if (2 * b + ia) * W_ACT // 16 != (2 * b + ia - 1) * W_ACT // 16:
                        nc.scalar.activation(
                            et[:, wsl], ps_b[:, o:o + 256], AF.Copy,
                            scale=u32[:, ia * C + b:ia * C + b + 1])
                    else:
                        nc.vector.tensor_scalar_mul(
                            et[:, wsl], ps_b[:, o:o + 256],
                            u32[:, ia * C + b:ia * C + b + 1])

            def _fn_mul(g, ch):
                # out halves: [128,1024] chunks -> fp16 into the it tile
                it, e0r, et = st[g]["it"], st[g]["e0r"], st[g]["et"]
                msl = slice(ch * 1024, (ch + 1) * 1024)
                if ch % (C // 2) < MUL_POOL:
                    nc.gpsimd.tensor_mul(it[:, msl], e0r[:, msl], et[:, msl])
                else:
                    nc.vector.tensor_mul(it[:, msl], e0r[:, msl], et[:, msl])

            def _st_half(g, h):
                it = st[g]["it"]
                hc = C // 2
                dst = out[g * C + h * hc:g * C + (h + 1) * hc].rearrange(
                    "b (h p) j -> p b h j", p=128)
                nc.sync.dma_start(
                    dst, it[:, h * hc * 512:(h + 1) * hc * 512].rearrange(
                        "p (b h j) -> p b h j", h=2, j=256))

            def emit_fn_pair(pa, pb, tr_units=(), next_gen=None):
                # interleave the two cohorts' final phases; weave the next
                # pair's remaining transpose units between early sample
                # groups, then start the NEXT pair's iteration phases
                _fn_pre(pa)
                _fn_pre(pb)
                wi = {pa: 0, pb: 0}
                done = {pa: 0, pb: 0}
                nu = len(tr_units)
                ti = 0
                for b in range(C):
                    if b % 2 == 0:
                        for g in (pa, pb):
                            _fn_bcast2(g, b // 2)
                    for g in (pa, pb):
                        _fn_sample(g, b, wi[g])
                        wi[g] += 2
                    want = min(nu, (b + 1) * nu // (C // 2))
                    while ti < want:
                        emit_tr_unit(*tr_units[ti])
                        ti += 1
                    if ti == nu and next_gen is not None:
                        next(next_gen, None)
                    # a [128,1024] mul covers 2 samples' w
                    if b % 2 == 1:
                        ch = b // 2
                        for g in (pa, pb):
                            _fn_mul(g, ch)

            pending_st = []
            gens = {}
            for s in range(P + 1):
                for g in pending_st:
                    _st_half(g, 0)
                    _st_half(g, 1)
                pending_st = []
                if s < P:
                    a, b = 2 * s, 2 * s + 1
                    emit_ld(a)
                    emit_ld(b)
                    emit_ex(a)
                    emit_ex(b)
                    emit_tr_alloc(a)
                    emit_tr_alloc(b)
                    units = [(g, sb) for sb in range(C) for g in (a, b)]
                    gens[s] = it_pair_gen(a, b)
                else:
                    units = []
                if s >= 1:
                    p = s - 1
                    pa, pb = 2 * p, 2 * p + 1
                    # finish IT(p): remaining phases woven with first half
                    # of TR(s) units
                    gen = gens[p]
                    nw = len(units) // 2
                    ti = 0
                    ph = 0
                    while True:
                        if next(gen, "END") == "END":
                            break
                        ph += 1
                        if ph > TR_WEAVE_FROM:
                            want = min(nw, (ph - TR_WEAVE_FROM) * nw // 4)
                            while ti < want:
                                emit_tr_unit(*units[ti])
                                ti += 1
                    emit_fn_pair(pa, pb, units[ti:], gens.get(s))
                    pending_st = [pa, pb]
                else:
                    for u in units:
                        emit_tr_unit(*u)

            for g in pending_st:
                _st_half(g, 0)
                _st_half(g, 1)

    nc.compile()
    return nc


def _get_nc():
    global _NC_CACHE
    if _NC_CACHE is None:
        _NC_CACHE = _build_nc()
    return _NC_CACHE


def _prep_in_maps(logits, free_agents_num, tasks_num):
    logits = np.asarray(logits, dtype=np.float32)
    free = np.asarray(free_agents_num).astype(np.int64)
    tasks = np.asarray(tasks_num).astype(np.int64)
    row_ok = np.arange(A, dtype=np.int64)[None, :] < free[:, None]   # [B, A]
    col_ok = np.arange(T, dtype=np.int64)[None, :] < tasks[:, None]  # [B, T]
    mask = row_ok[:, :, None] & col_ok[:, None, :]
    lgm = np.where(mask, logits, np.float32(MASKVAL)).astype(np.float16)
    ident = np.eye(128, dtype=np.float32)
    identsc = (ident * OUT_SCALE).astype(np.float32)
    sel = np.zeros((C, C * 128), dtype=np.float32)
    for b in range(C):
        sel[b, b * 128:(b + 1) * 128] = 1.0
    consts = np.zeros((1, 256), dtype=np.float32)
    consts[0, 0:128] = EPS
    consts[0, 128:128 + 4 * C] = 1.0
    return [
        {
            "lg": np.ascontiguousarray(lgm[c * BPC:(c + 1) * BPC]),
            "ident": ident,
            "identsc": identsc,
            "sel": sel,
            "consts": consts,
            "onesd": np.ones((128, 4 * C), dtype=np.float32),
        }
        for c in range(NCORES)
    ]


def _run(logits, free_agents_num, tasks_num, **spmd_kwargs):
    from concourse.bass_utils import run_bass_kernel_spmd

    in_maps = _prep_in_maps(logits, free_agents_num, tasks_num)
    res = run_bass_kernel_spmd(
        _get_nc(), in_maps, core_ids=list(range(NCORES)), **spmd_kwargs
    )
    out = np.concatenate(
        [r["out"].astype(np.float32) for r in res.results], axis=0)
    return out, res


def kernel(logits, free_agents_num, tasks_num):
    out, _ = _run(logits, free_agents_num, tasks_num)
    return out


# revision 30
# speedup vs baseline: 1.1192x; 1.1192x over previous
"""Gumbel-Sinkhorn (masked, 5 iterations) on Trainium2, data-parallel over 8 cores.

Math: the reference's masked log-domain Sinkhorn equals, in the probability
domain, classic Sinkhorn scaling of K = exp(masked_logits):

    v_0 = 1;  u_k = 1/(K v_{k-1} + eps);  v_k = 1/(K^T u_k + eps)   (k = 1..5)
    out = K * (u_5 outer v_5) * exp(1e-6),   masked entries exactly 0.

V2: fp16 HBM I/O (halves DMA, the roofline term), float32r compute on-chip.
K^T is built with PE transposes instead of loading a host-transposed copy
(v1 spent 16 MiB/core of HBM traffic on that). The eps guard rides on the PE
as a rank-1 PSUM accumulate, so no DVE max pass. Final out = K*(u x v):
PE broadcasts v-rows per sample into PSUM, ACT/DVE apply the per-partition
u scale (w = ps_b * u, written into the dead et tile), DVE/Pool multiply.

Emission is software-pipelined over cohort PAIRS so the per-engine in-order
streams interleave across cohorts: while pair p iterates (PE/DVE ping-pong),
ACT runs pair p+1's exp and the final-phase scale-copies of pair p-1, Pool
multiplies pair p-1, DMA streams pair p+1 in / p-1 out. Within a pair the
ten matvec-reciprocal phases alternate cohorts A/B so each engine's
dependency stalls are filled by the sibling cohort.

Host-side prep (cheap numpy): mask logits to -1e4, cast fp16; output is cast
back to fp32. All O(B*A*T) math runs on device.
"""

import numpy as np

B, A, T = 512, 256, 256
NCORES = 8
BPC = B // NCORES          # samples per core
C = 8                      # cohort size (samples in lockstep)
G = BPC // C               # cohorts per core
P = G // 2                 # cohort pairs (pipeline unit)
ITERS = 5
MASKVAL = np.float16(-1e4)  # exp(-1e4) == 0.0 exactly
EPS = 1e-18                 # rank-1 PSUM bias; guards 1/0 on masked rows/cols.
                            # Added (not max'ed) into every row/col sum, so it must
                            # sit far below the smallest valid sum (~1e-13) while
                            # keeping 1/EPS^2 = 1e36 finite in fp32 for fully-masked
                            # row x column pairs in the u*v broadcast.
OUT_SCALE = float(np.exp(np.float64(1e-6)))  # reference's exp(x + 1e-6)

# --- engine-assignment knobs (per cohort) ---
ET_ACT = 7     # of the 8 et-drain chunks per cohort, how many go to ACT
W_ACT = 5      # of the 16 w scale-copies per cohort, how many go to ACT
MUL_POOL = 3   # of the 4 [128,1024] mul chunks per cohort on Pool (rest DVE)
TR_WEAVE_FROM = 4  # first iteration phase that carries transpose units
UNIT_SPREAD = 4    # FN sample-groups over which leftover TR units spread
PH_PER_GROUP = 1   # next-pair IT phases woven per FN sample-group

_NC_CACHE = None


def _build_nc():
    import concourse.tile as tile
    from concourse import bacc, mybir

    f32 = mybir.dt.float32
    f32r = mybir.dt.float32r
    fp16 = mybir.dt.float16
    AF = mybir.ActivationFunctionType

    nc = bacc.Bacc()
    lg = nc.dram_tensor("lg", [BPC, A, T], fp16, kind="ExternalInput")
    ident = nc.dram_tensor("ident", [128, 128], f32r, kind="ExternalInput")
    identsc = nc.dram_tensor("identsc", [128, 128], f32r, kind="ExternalInput")
    # sel[k, b*128+m] = 1 if k == b else 0 (v-row -> per-sample broadcast)
    sel = nc.dram_tensor("sel", [C, C * 128], f32r, kind="ExternalInput")
    # consts[0, 0:128] = EPS; consts[0, 128:128+8C] = 1.0
    consts = nc.dram_tensor("consts", [1, 256], f32r, kind="ExternalInput")
    onesd = nc.dram_tensor("onesd", [128, 8 * C], f32r, kind="ExternalInput")
    out = nc.dram_tensor("out", [BPC, A, T], fp16, kind="ExternalOutput")

    SLAB = C * 512  # free elems per cohort slab (per sample: 2 halves x 256)

    with tile.TileContext(nc) as tc:
        with (
            tc.tile_pool(name="itp", bufs=6) as itp,
            tc.tile_pool(name="e0p", bufs=5) as e0p,
            tc.tile_pool(name="etp", bufs=4) as etp,
            tc.tile_pool(name="uvp", bufs=10) as uvp,
            tc.tile_pool(name="vrowp", bufs=2) as vrowp,
            tc.tile_pool(name="constp", bufs=1) as constp,
            tc.tile_pool(name="pstp", bufs=3, space="PSUM") as pstp,
            tc.tile_pool(name="psuv", bufs=2, space="PSUM") as psuv,
            tc.tile_pool(name="psb", bufs=3, space="PSUM") as psbp,
        ):
            idr = constp.tile([128, 128], f32r)
            nc.sync.dma_start(idr[:], ident[:])
            idsc = constp.tile([128, 128], f32r)
            nc.sync.dma_start(idsc[:], identsc[:])
            sel_sb = constp.tile([C, C * 128], f32r)
            nc.sync.dma_start(sel_sb[0:C, :], sel[:])
            cst = constp.tile([1, 256], f32r)
            nc.sync.dma_start(cst[:], consts[:])
            ones2 = constp.tile([128, 8 * C], f32r)
            nc.sync.dma_start(ones2[:], onesd[:])

            st = {}  # per-cohort pipeline state: tiles

            def emit_ld(g):
                it = itp.tile([128, SLAB], fp16, name="it")
                hc = C // 2
                for h in range(2):
                    src = lg[g * C + h * hc:g * C + (h + 1) * hc].rearrange(
                        "b (h p) j -> p b h j", p=128)
                    nc.sync.dma_start(
                        it[:, h * hc * 512:(h + 1) * hc * 512].rearrange(
                            "p (b h j) -> p b h j", h=2, j=256), src)
                st[g] = {"it": it}

            def emit_ex(g):
                it = st[g]["it"]
                e0r = e0p.tile([128, SLAB], f32r, name="e0r")
                for h in range(2):
                    sl = slice(h * SLAB // 2, (h + 1) * SLAB // 2)
                    nc.scalar.activation(e0r[:, sl], it[:, sl], AF.Exp)
                st[g]["e0r"] = e0r

            def emit_tr_unit(g, b):
                # et[p=j', b*512+jt*256+ia*128+i'] = K[b, ia*128+i', jt*128+j']
                e0r, et = st[g]["e0r"], st[g]["et"]
                pst = pstp.tile([128, 512], f32r, name="pst")
                for ia in range(2):
                    for jt in range(2):
                        nc.tensor.transpose(
                            pst[:, jt * 256 + ia * 128:
                                jt * 256 + ia * 128 + 128],
                            e0r[:, b * 512 + ia * 256 + jt * 128:
                                b * 512 + ia * 256 + jt * 128 + 128],
                            idr[:],
                        )
                dst = et[:, b * 512:(b + 1) * 512]
                if b < ET_ACT:
                    nc.scalar.activation(dst, pst[:], AF.Copy)
                else:
                    nc.vector.tensor_copy(dst, pst[:])

            def emit_tr_alloc(g):
                st[g]["et"] = etp.tile([128, SLAB], f32r, name="et")

            def _phase_u(g):
                et = st[g]["et"]
                v2t, v2o = st[g]["v2"]
                ps_u = psuv.tile([128, 4 * C], f32, name="ps_uv")
                nc.tensor.matmul(ps_u[:], lhsT=cst[:, 0:128],
                                 rhs=cst[:, 128:128 + 4 * C],
                                 start=True, stop=False, skip_group_check=True)
                for b in range(C):
                    for ia in range(2):
                        col = 2 * (ia * C + b)
                        for jt in range(2):
                            nc.tensor.matmul(
                                ps_u[:, col:col + 2],
                                lhsT=et[:, b * 512 + jt * 256 + ia * 128:
                                        b * 512 + jt * 256 + ia * 128 + 128],
                                rhs=v2t[:, v2o + 2 * (jt * C + b):
                                        v2o + 2 * (jt * C + b) + 2],
                                start=False, stop=(jt == 1),
                                skip_group_check=True,
                            )
                u2 = uvp.tile([128, 4 * C], f32r, name="u2")
                with nc.allow_low_precision(reason="f32r scale vectors"):
                    nc.vector.reciprocal(
                        u2[:].rearrange("p (c d) -> p c d", d=2),
                        ps_u[:].rearrange("p (c d) -> p c d", d=2)
                        [:, :, 0:1].broadcast_to([128, 2 * C, 2]))
                st[g]["u2"] = (u2, 0)

            def _phase_v(g):
                e0r = st[g]["e0r"]
                u2t, u2o = st[g]["u2"]
                ps_v = psuv.tile([128, 4 * C], f32, name="ps_uv")
                nc.tensor.matmul(ps_v[:], lhsT=cst[:, 0:128],
                                 rhs=cst[:, 128:128 + 4 * C],
                                 start=True, stop=False, skip_group_check=True)
                for b in range(C):
                    for jt in range(2):
                        col = 2 * (jt * C + b)
                        for ia in range(2):
                            nc.tensor.matmul(
                                ps_v[:, col:col + 2],
                                lhsT=e0r[:, b * 512 + ia * 256 + jt * 128:
                                         b * 512 + ia * 256 + jt * 128 + 128],
                                rhs=u2t[:, u2o + 2 * (ia * C + b):
                                        u2o + 2 * (ia * C + b) + 2],
                                start=False, stop=(ia == 1),
                                skip_group_check=True,
                            )
                v2 = uvp.tile([128, 4 * C], f32r, name="v2")
                with nc.allow_low_precision(reason="f32r scale vectors"):
                    nc.vector.reciprocal(
                        v2[:].rearrange("p (c d) -> p c d", d=2),
                        ps_v[:].rearrange("p (c d) -> p c d", d=2)
                        [:, :, 0:1].broadcast_to([128, 2 * C, 2]))
                st[g]["v2"] = (v2, 0)

            def it_pair_gen(a, b):
                # yields once per phase-half (10 yields for 5 iterations);
                # the scheduler weaves other work between phase-halves
                for g in (a, b):
                    st[g]["v2"] = (ones2, 0)
                for itn in range(ITERS):
                    for half in range(2):
                        for g in (a, b):
                            if half == 0:
                                _phase_u(g)
                            else:
                                _phase_v(g)
                        yield

            def _fn_pre(g):
                u2t, u2o = st[g]["u2"]
                v2t, v2o = st[g]["v2"]
                u32 = uvp.tile([128, 2 * C], f32, name="u32")
                nc.vector.tensor_copy(u32[:], u2t[:, u2o:u2o + 4 * C:2])
                ps_vr = psbp.tile([C, 256], f32r, name="ps_b")
                for jt in range(2):
                    nc.tensor.transpose(
                        ps_vr[0:C, jt * 128:(jt + 1) * 128],
                        v2t[:, v2o + 2 * jt * C:v2o + 2 * (jt + 1) * C:2],
                        idsc[:],
                    )
                vrow = vrowp.tile([C, 256], f32r, name="vrow")
                nc.vector.tensor_copy(vrow[0:C, :], ps_vr[0:C, :])
                st[g]["u32"], st[g]["vrow"] = u32, vrow

            def _fn_bcast2(g, b2):
                # one psum bank holds two samples' v-row broadcasts
                vrow = st[g]["vrow"]
                ps_b = psbp.tile([128, 512], f32, name="ps_b")
                for o in range(2):
                    b = 2 * b2 + o
                    nc.tensor.matmul(
                        ps_b[:, o * 256:(o + 1) * 256],
                        lhsT=sel_sb[0:C, b * 128:(b + 1) * 128],
                        rhs=vrow[0:C, :], start=True, stop=True,
                    )
                st[g].setdefault("ps_b", {})[b2] = ps_b

            def _fn_sample(g, b, wi):
                # w = ps_b * u for sample b, written into the dead et tile
                et, u32 = st[g]["et"], st[g]["u32"]
                ps_b = st[g]["ps_b"][b // 2]
                o = (b % 2) * 256
                for ia in range(2):
                    wsl = slice(b * 512 + ia * 256, b * 512 + (ia + 1) * 256)
                    if (2 * b + ia) * W_ACT // 16 != (2 * b + ia - 1) * W_ACT // 16:
                        nc.scalar.activation(
                            et[:, wsl], ps_b[:, o:o + 256], AF.Copy,
                            scale=u32[:, ia * C + b:ia * C + b + 1])
                    else:
                        nc.vector.tensor_scalar_mul(
                            et[:, wsl], ps_b[:, o:o + 256],
                            u32[:, ia * C + b:ia * C + b + 1])

            def _fn_mul(g, ch):
                # out halves: [128,1024] chunks -> fp16 into the it tile
                it, e0r, et = st[g]["it"], st[g]["e0r"], st[g]["et"]
                msl = slice(ch * 1024, (ch + 1) * 1024)
                if ch % (C // 2) < MUL_POOL:
                    nc.gpsimd.tensor_mul(it[:, msl], e0r[:, msl], et[:, msl])
                else:
                    nc.vector.tensor_mul(it[:, msl], e0r[:, msl], et[:, msl])

            def _st_half(g, h):
                it = st[g]["it"]
                hc = C // 2
                dst = out[g * C + h * hc:g * C + (h + 1) * hc].rearrange(
                    "b (h p) j -> p b h j", p=128)
                nc.sync.dma_start(
                    dst, it[:, h * hc * 512:(h + 1) * hc * 512].rearrange(
                        "p (b h j) -> p b h j", h=2, j=256))

            def emit_fn_pair(pa, pb, tr_units=(), next_gen=None):
                # interleave the two cohorts' final phases; weave the next
                # pair's remaining transpose units between early sample
                # groups, then start the NEXT pair's iteration phases
                _fn_pre(pa)
                _fn_pre(pb)
                wi = {pa: 0, pb: 0}
                done = {pa: 0, pb: 0}
                nu = len(tr_units)
                ti = 0
                for b in range(C):
                    if b % 2 == 0:
                        for g in (pa, pb):
                            _fn_bcast2(g, b // 2)
                    for g in (pa, pb):
                        _fn_sample(g, b, wi[g])
                        wi[g] += 2
                    want = min(nu, (b + 1) * nu // UNIT_SPREAD)
                    while ti < want:
                        emit_tr_unit(*tr_units[ti])
                        ti += 1
                    if ti == nu and next_gen is not None:
                        for _ in range(PH_PER_GROUP):
                            next(next_gen, None)
                    # a [128,1024] mul covers 2 samples' w
                    if b % 2 == 1:
                        ch = b // 2
                        for g in (pa, pb):
                            _fn_mul(g, ch)

            pending_st = []
            gens = {}
            for s in range(P + 1):
                for g in pending_st:
                    _st_half(g, 0)
                    _st_half(g, 1)
                pending_st = []
                if s < P:
                    a, b = 2 * s, 2 * s + 1
                    emit_ld(a)
                    emit_ld(b)
                    emit_ex(a)
                    emit_ex(b)
                    emit_tr_alloc(a)
                    emit_tr_alloc(b)
                    units = [(g, sb) for sb in range(C) for g in (a, b)]
                    gens[s] = it_pair_gen(a, b)
                else:
                    units = []
                if s >= 1:
                    p = s - 1
                    pa, pb = 2 * p, 2 * p + 1
                    # finish IT(p): remaining phases woven with first half
                    # of TR(s) units
                    gen = gens[p]
                    nw = len(units) // 2
                    ti = 0
                    ph = 0
                    while True:
                        if next(gen, "END") == "END":
                            break
                        ph += 1
                        if ph > TR_WEAVE_FROM:
                            want = min(nw, (ph - TR_WEAVE_FROM) * nw // 4)
                            while ti < want:
                                emit_tr_unit(*units[ti])
                                ti += 1
                    emit_fn_pair(pa, pb, units[ti:], gens.get(s))
                    pending_st = [pa, pb]
                else:
                    for u in units:
                        emit_tr_unit(*u)

            for g in pending_st:
                _st_half(g, 0)
                _st_half(g, 1)

    nc.compile()
    return nc


def _get_nc():
    global _NC_CACHE
    if _NC_CACHE is None:
        _NC_CACHE = _build_nc()
    return _NC_CACHE


def _prep_in_maps(logits, free_agents_num, tasks_num):
    logits = np.asarray(logits, dtype=np.float32)
    free = np.asarray(free_agents_num).astype(np.int64)
    tasks = np.asarray(tasks_num).astype(np.int64)
    row_ok = np.arange(A, dtype=np.int64)[None, :] < free[:, None]   # [B, A]
    col_ok = np.arange(T, dtype=np.int64)[None, :] < tasks[:, None]  # [B, T]
    mask = row_ok[:, :, None] & col_ok[:, None, :]
    lgm = np.where(mask, logits, np.float32(MASKVAL)).astype(np.float16)
    ident = np.eye(128, dtype=np.float32)
    identsc = (ident * OUT_SCALE).astype(np.float32)
    sel = np.zeros((C, C * 128), dtype=np.float32)
    for b in range(C):
        sel[b, b * 128:(b + 1) * 128] = 1.0
    consts = np.zeros((1, 256), dtype=np.float32)
    consts[0, 0:128] = EPS
    consts[0, 128:128 + 8 * C] = 1.0
    return [
        {
            "lg": np.ascontiguousarray(lgm[c * BPC:(c + 1) * BPC]),
            "ident": ident,
            "identsc": identsc,
            "sel": sel,
            "consts": consts,
            "onesd": np.ones((128, 8 * C), dtype=np.float32),
        }
        for c in range(NCORES)
    ]


def _run(logits, free_agents_num, tasks_num, **spmd_kwargs):
    from concourse.bass_utils import run_bass_kernel_spmd

    in_maps = _prep_in_maps(logits, free_agents_num, tasks_num)
    res = run_bass_kernel_spmd(
        _get_nc(), in_maps, core_ids=list(range(NCORES)), **spmd_kwargs
    )
    out = np.concatenate(
        [r["out"].astype(np.float32) for r in res.results], axis=0)
    return out, res


def kernel(logits, free_agents_num, tasks_num):
    out, _ = _run(logits, free_agents_num, tasks_num)
    return out


# revision 31
# speedup vs baseline: 1.1262x; 1.0063x over previous
"""Gumbel-Sinkhorn (masked, 5 iterations) on Trainium2, data-parallel over 8 cores.

Math: the reference's masked log-domain Sinkhorn equals, in the probability
domain, classic Sinkhorn scaling of K = exp(masked_logits):

    v_0 = 1;  u_k = 1/(K v_{k-1} + eps);  v_k = 1/(K^T u_k + eps)   (k = 1..5)
    out = K * (u_5 outer v_5) * exp(1e-6),   masked entries exactly 0.

V2: fp16 HBM I/O (halves DMA, the roofline term), float32r compute on-chip.
K^T is built with PE transposes instead of loading a host-transposed copy
(v1 spent 16 MiB/core of HBM traffic on that). The eps guard rides on the PE
as a rank-1 PSUM accumulate, so no DVE max pass. Final out = K*(u x v):
PE broadcasts v-rows per sample into PSUM, ACT/DVE apply the per-partition
u scale (w = ps_b * u, written into the dead et tile), DVE/Pool multiply.

Emission is software-pipelined over cohort PAIRS so the per-engine in-order
streams interleave across cohorts: while pair p iterates (PE/DVE ping-pong),
ACT runs pair p+1's exp and the final-phase scale-copies of pair p-1, Pool
multiplies pair p-1, DMA streams pair p+1 in / p-1 out. Within a pair the
ten matvec-reciprocal phases alternate cohorts A/B so each engine's
dependency stalls are filled by the sibling cohort.

Host-side prep (cheap numpy): mask logits to -1e4, cast fp16; output is cast
back to fp32. All O(B*A*T) math runs on device.
"""

import numpy as np

B, A, T = 512, 256, 256
NCORES = 8
BPC = B // NCORES          # samples per core
C = 8                      # cohort size (samples in lockstep)
G = BPC // C               # cohorts per core
P = G // 2                 # cohort pairs (pipeline unit)
ITERS = 5
MASKVAL = np.float16(-1e4)  # exp(-1e4) == 0.0 exactly
EPS = 1e-18                 # rank-1 PSUM bias; guards 1/0 on masked rows/cols.
                            # Added (not max'ed) into every row/col sum, so it must
                            # sit far below the smallest valid sum (~1e-13) while
                            # keeping 1/EPS^2 = 1e36 finite in fp32 for fully-masked
                            # row x column pairs in the u*v broadcast.
OUT_SCALE = float(np.exp(np.float64(1e-6)))  # reference's exp(x + 1e-6)

# --- engine-assignment knobs (per cohort) ---
ET_ACT = 7     # of the 8 et-drain chunks per cohort, how many go to ACT
W_ACT = 5      # of the 16 w scale-copies per cohort, how many go to ACT
MUL_POOL = 3   # of the 4 [128,1024] mul chunks per cohort on Pool (rest DVE)
TR_WEAVE_FROM = 4  # first iteration phase that carries transpose units
UNIT_SPREAD = 4    # FN sample-groups over which leftover TR units spread
PH_PER_GROUP = 1   # next-pair IT phases woven per FN sample-group

_NC_CACHE = None


def _build_nc():
    import concourse.tile as tile
    from concourse import bacc, mybir

    f32 = mybir.dt.float32
    f32r = mybir.dt.float32r
    fp16 = mybir.dt.float16
    AF = mybir.ActivationFunctionType

    nc = bacc.Bacc()
    lg = nc.dram_tensor("lg", [BPC, A, T], fp16, kind="ExternalInput")
    ident = nc.dram_tensor("ident", [128, 128], f32r, kind="ExternalInput")
    identsc = nc.dram_tensor("identsc", [128, 128], f32r, kind="ExternalInput")
    # sel[k, b*128+m] = 1 if k == b else 0 (v-row -> per-sample broadcast)
    sel = nc.dram_tensor("sel", [C, C * 128], f32r, kind="ExternalInput")
    # consts[0, 0:128] = EPS; consts[0, 128:128+8C] = 1.0
    consts = nc.dram_tensor("consts", [1, 256], f32r, kind="ExternalInput")
    onesd = nc.dram_tensor("onesd", [128, 8 * C], f32r, kind="ExternalInput")
    out = nc.dram_tensor("out", [BPC, A, T], fp16, kind="ExternalOutput")

    SLAB = C * 512  # free elems per cohort slab (per sample: 2 halves x 256)

    with tile.TileContext(nc) as tc:
        with (
            tc.tile_pool(name="itp", bufs=6) as itp,
            tc.tile_pool(name="e0p", bufs=5) as e0p,
            tc.tile_pool(name="etp", bufs=4) as etp,
            tc.tile_pool(name="uvp", bufs=10) as uvp,
            tc.tile_pool(name="vrowp", bufs=2) as vrowp,
            tc.tile_pool(name="constp", bufs=1) as constp,
            tc.tile_pool(name="pstp", bufs=3, space="PSUM") as pstp,
            tc.tile_pool(name="psuv", bufs=2, space="PSUM") as psuv,
            tc.tile_pool(name="psb", bufs=3, space="PSUM") as psbp,
        ):
            idr = constp.tile([128, 128], f32r)
            nc.sync.dma_start(idr[:], ident[:])
            idsc = constp.tile([128, 128], f32r)
            nc.sync.dma_start(idsc[:], identsc[:])
            sel_sb = constp.tile([C, C * 128], f32r)
            nc.sync.dma_start(sel_sb[0:C, :], sel[:])
            cst = constp.tile([1, 256], f32r)
            nc.sync.dma_start(cst[:], consts[:])
            ones2 = constp.tile([128, 8 * C], f32r)
            nc.sync.dma_start(ones2[:], onesd[:])

            st = {}  # per-cohort pipeline state: tiles

            def emit_ld(g):
                it = itp.tile([128, SLAB], fp16, name="it")
                hc = C // 2
                for h in range(2):
                    src = lg[g * C + h * hc:g * C + (h + 1) * hc].rearrange(
                        "b (h p) j -> p b h j", p=128)
                    nc.sync.dma_start(
                        it[:, h * hc * 512:(h + 1) * hc * 512].rearrange(
                            "p (b h j) -> p b h j", h=2, j=256), src)
                st[g] = {"it": it}

            def emit_ex(g):
                it = st[g]["it"]
                e0r = e0p.tile([128, SLAB], f32r, name="e0r")
                for h in range(2):
                    sl = slice(h * SLAB // 2, (h + 1) * SLAB // 2)
                    nc.scalar.activation(e0r[:, sl], it[:, sl], AF.Exp)
                st[g]["e0r"] = e0r

            def emit_tr_unit(g, b):
                # et[p=j', b*512+jt*256+ia*128+i'] = K[b, ia*128+i', jt*128+j']
                e0r, et = st[g]["e0r"], st[g]["et"]
                pst = pstp.tile([128, 512], f32r, name="pst")
                for ia in range(2):
                    for jt in range(2):
                        nc.tensor.transpose(
                            pst[:, jt * 256 + ia * 128:
                                jt * 256 + ia * 128 + 128],
                            e0r[:, b * 512 + ia * 256 + jt * 128:
                                b * 512 + ia * 256 + jt * 128 + 128],
                            idr[:],
                        )
                dst = et[:, b * 512:(b + 1) * 512]
                if b < ET_ACT:
                    nc.scalar.activation(dst, pst[:], AF.Copy)
                else:
                    nc.vector.tensor_copy(dst, pst[:])

            def emit_tr_alloc(g):
                st[g]["et"] = etp.tile([128, SLAB], f32r, name="et")

            def _phase_u(g):
                et = st[g]["et"]
                v2t, v2o = st[g]["v2"]
                ps_u = psuv.tile([128, 4 * C], f32, name="ps_uv")
                nc.tensor.matmul(ps_u[:], lhsT=cst[:, 0:128],
                                 rhs=cst[:, 128:128 + 4 * C],
                                 start=True, stop=False, skip_group_check=True)
                for b in range(C):
                    for ia in range(2):
                        col = 2 * (ia * C + b)
                        for jt in range(2):
                            nc.tensor.matmul(
                                ps_u[:, col:col + 2],
                                lhsT=et[:, b * 512 + jt * 256 + ia * 128:
                                        b * 512 + jt * 256 + ia * 128 + 128],
                                rhs=v2t[:, v2o + 2 * (jt * C + b):
                                        v2o + 2 * (jt * C + b) + 2],
                                start=False, stop=(jt == 1),
                                skip_group_check=True,
                            )
                u2 = uvp.tile([128, 4 * C], f32r, name="u2")
                with nc.allow_low_precision(reason="f32r scale vectors"):
                    nc.vector.reciprocal(
                        u2[:].rearrange("p (c d) -> p c d", d=2),
                        ps_u[:].rearrange("p (c d) -> p c d", d=2)
                        [:, :, 0:1].broadcast_to([128, 2 * C, 2]))
                st[g]["u2"] = (u2, 0)

            def _phase_v(g):
                e0r = st[g]["e0r"]
                u2t, u2o = st[g]["u2"]
                ps_v = psuv.tile([128, 4 * C], f32, name="ps_uv")
                nc.tensor.matmul(ps_v[:], lhsT=cst[:, 0:128],
                                 rhs=cst[:, 128:128 + 4 * C],
                                 start=True, stop=False, skip_group_check=True)
                for b in range(C):
                    for jt in range(2):
                        col = 2 * (jt * C + b)
                        for ia in range(2):
                            nc.tensor.matmul(
                                ps_v[:, col:col + 2],
                                lhsT=e0r[:, b * 512 + ia * 256 + jt * 128:
                                         b * 512 + ia * 256 + jt * 128 + 128],
                                rhs=u2t[:, u2o + 2 * (ia * C + b):
                                        u2o + 2 * (ia * C + b) + 2],
                                start=False, stop=(ia == 1),
                                skip_group_check=True,
                            )
                v2 = uvp.tile([128, 4 * C], f32r, name="v2")
                with nc.allow_low_precision(reason="f32r scale vectors"):
                    nc.vector.reciprocal(
                        v2[:].rearrange("p (c d) -> p c d", d=2),
                        ps_v[:].rearrange("p (c d) -> p c d", d=2)
                        [:, :, 0:1].broadcast_to([128, 2 * C, 2]))
                st[g]["v2"] = (v2, 0)

            def it_pair_gen(a, b):
                # yields once per phase-half (10 yields for 5 iterations);
                # the scheduler weaves other work between phase-halves
                for g in (a, b):
                    st[g]["v2"] = (ones2, 0)
                for itn in range(ITERS):
                    for half in range(2):
                        for g in (a, b):
                            if half == 0:
                                _phase_u(g)
                            else:
                                _phase_v(g)
                        yield

            def _fn_pre(g):
                u2t, u2o = st[g]["u2"]
                v2t, v2o = st[g]["v2"]
                u32 = uvp.tile([128, 2 * C], f32, name="u32")
                nc.vector.tensor_copy(u32[:], u2t[:, u2o:u2o + 4 * C:2])
                ps_vr = psbp.tile([C, 256], f32r, name="ps_b")
                for jt in range(2):
                    nc.tensor.transpose(
                        ps_vr[0:C, jt * 128:(jt + 1) * 128],
                        v2t[:, v2o + 2 * jt * C:v2o + 2 * (jt + 1) * C:2],
                        idsc[:],
                    )
                vrow = vrowp.tile([C, 256], f32r, name="vrow")
                nc.vector.tensor_copy(vrow[0:C, :], ps_vr[0:C, :])
                st[g]["u32"], st[g]["vrow"] = u32, vrow

            def _fn_bcast2(g, b2):
                # one psum bank holds two samples' v-row broadcasts
                vrow = st[g]["vrow"]
                ps_b = psbp.tile([128, 512], f32, name="ps_b")
                for o in range(2):
                    b = 2 * b2 + o
                    nc.tensor.matmul(
                        ps_b[:, o * 256:(o + 1) * 256],
                        lhsT=sel_sb[0:C, b * 128:(b + 1) * 128],
                        rhs=vrow[0:C, :], start=True, stop=True,
                    )
                st[g].setdefault("ps_b", {})[b2] = ps_b

            def _fn_sample(g, b, wi):
                # w = ps_b * u for sample b, written into the dead et tile
                et, u32 = st[g]["et"], st[g]["u32"]
                ps_b = st[g]["ps_b"][b // 2]
                o = (b % 2) * 256
                for ia in range(2):
                    wsl = slice(b * 512 + ia * 256, b * 512 + (ia + 1) * 256)
                    if (2 * b + ia) * W_ACT // 16 != (2 * b + ia - 1) * W_ACT // 16:
                        nc.scalar.activation(
                            et[:, wsl], ps_b[:, o:o + 256], AF.Copy,
                            scale=u32[:, ia * C + b:ia * C + b + 1])
                    else:
                        nc.vector.tensor_scalar_mul(
                            et[:, wsl], ps_b[:, o:o + 256],
                            u32[:, ia * C + b:ia * C + b + 1])

            def _fn_mul(g, ch):
                # out halves: [128,1024] chunks -> fp16 into the it tile.
                # The drain-tail pair splits muls evenly so idle DVE helps.
                it, e0r, et = st[g]["it"], st[g]["e0r"], st[g]["et"]
                msl = slice(ch * 1024, (ch + 1) * 1024)
                mp = last_mp[0] if last_mp[0] is not None else MUL_POOL
                if ch % (C // 2) < mp:
                    nc.gpsimd.tensor_mul(it[:, msl], e0r[:, msl], et[:, msl])
                else:
                    nc.vector.tensor_mul(it[:, msl], e0r[:, msl], et[:, msl])

            def _st_half(g, h):
                it = st[g]["it"]
                hc = C // 2
                dst = out[g * C + h * hc:g * C + (h + 1) * hc].rearrange(
                    "b (h p) j -> p b h j", p=128)
                nc.sync.dma_start(
                    dst, it[:, h * hc * 512:(h + 1) * hc * 512].rearrange(
                        "p (b h j) -> p b h j", h=2, j=256))

            def emit_fn_pair(pa, pb, tr_units=(), next_gen=None):
                # interleave the two cohorts' final phases; weave the next
                # pair's remaining transpose units between early sample
                # groups, then start the NEXT pair's iteration phases
                _fn_pre(pa)
                _fn_pre(pb)
                wi = {pa: 0, pb: 0}
                done = {pa: 0, pb: 0}
                nu = len(tr_units)
                ti = 0
                for b in range(C):
                    if b % 2 == 0:
                        for g in (pa, pb):
                            _fn_bcast2(g, b // 2)
                    for g in (pa, pb):
                        _fn_sample(g, b, wi[g])
                        wi[g] += 2
                    want = min(nu, (b + 1) * nu // UNIT_SPREAD)
                    while ti < want:
                        emit_tr_unit(*tr_units[ti])
                        ti += 1
                    if ti == nu and next_gen is not None:
                        for _ in range(PH_PER_GROUP):
                            next(next_gen, None)
                    # a [128,1024] mul covers 2 samples' w
                    if b % 2 == 1:
                        ch = b // 2
                        for g in (pa, pb):
                            _fn_mul(g, ch)

            pending_st = []
            last_mp = [None]
            gens = {}
            for s in range(P + 1):
                for g in pending_st:
                    _st_half(g, 0)
                    _st_half(g, 1)
                pending_st = []
                if s < P:
                    a, b = 2 * s, 2 * s + 1
                    emit_ld(a)
                    emit_ld(b)
                    emit_ex(a)
                    emit_ex(b)
                    emit_tr_alloc(a)
                    emit_tr_alloc(b)
                    units = [(g, sb) for sb in range(C) for g in (a, b)]
                    gens[s] = it_pair_gen(a, b)
                else:
                    units = []
                if s >= 1:
                    p = s - 1
                    if s == P:
                        last_mp[0] = 2
                    pa, pb = 2 * p, 2 * p + 1
                    # finish IT(p): remaining phases woven with first half
                    # of TR(s) units
                    gen = gens[p]
                    nw = len(units) // 2
                    ti = 0
                    ph = 0
                    while True:
                        if next(gen, "END") == "END":
                            break
                        ph += 1
                        if ph > TR_WEAVE_FROM:
                            want = min(nw, (ph - TR_WEAVE_FROM) * nw // 4)
                            while ti < want:
                                emit_tr_unit(*units[ti])
                                ti += 1
                    emit_fn_pair(pa, pb, units[ti:], gens.get(s))
                    pending_st = [pa, pb]
                else:
                    for u in units:
                        emit_tr_unit(*u)

            for g in pending_st:
                _st_half(g, 0)
                _st_half(g, 1)

    nc.compile()
    return nc


def _get_nc():
    global _NC_CACHE
    if _NC_CACHE is None:
        _NC_CACHE = _build_nc()
    return _NC_CACHE


def _prep_in_maps(logits, free_agents_num, tasks_num):
    logits = np.asarray(logits, dtype=np.float32)
    free = np.asarray(free_agents_num).astype(np.int64)
    tasks = np.asarray(tasks_num).astype(np.int64)
    row_ok = np.arange(A, dtype=np.int64)[None, :] < free[:, None]   # [B, A]
    col_ok = np.arange(T, dtype=np.int64)[None, :] < tasks[:, None]  # [B, T]
    mask = row_ok[:, :, None] & col_ok[:, None, :]
    lgm = np.where(mask, logits, np.float32(MASKVAL)).astype(np.float16)
    ident = np.eye(128, dtype=np.float32)
    identsc = (ident * OUT_SCALE).astype(np.float32)
    sel = np.zeros((C, C * 128), dtype=np.float32)
    for b in range(C):
        sel[b, b * 128:(b + 1) * 128] = 1.0
    consts = np.zeros((1, 256), dtype=np.float32)
    consts[0, 0:128] = EPS
    consts[0, 128:128 + 8 * C] = 1.0
    return [
        {
            "lg": np.ascontiguousarray(lgm[c * BPC:(c + 1) * BPC]),
            "ident": ident,
            "identsc": identsc,
            "sel": sel,
            "consts": consts,
            "onesd": np.ones((128, 8 * C), dtype=np.float32),
        }
        for c in range(NCORES)
    ]


def _run(logits, free_agents_num, tasks_num, **spmd_kwargs):
    from concourse.bass_utils import run_bass_kernel_spmd

    in_maps = _prep_in_maps(logits, free_agents_num, tasks_num)
    res = run_bass_kernel_spmd(
        _get_nc(), in_maps, core_ids=list(range(NCORES)), **spmd_kwargs
    )
    out = np.concatenate(
        [r["out"].astype(np.float32) for r in res.results], axis=0)
    return out, res


def kernel(logits, free_agents_num, tasks_num):
    out, _ = _run(logits, free_agents_num, tasks_num)
    return out


# revision 32
# speedup vs baseline: 1.1717x; 1.0404x over previous
"""Gumbel-Sinkhorn (masked, 5 iterations) on Trainium2, data-parallel over 8 cores.

Math: the reference's masked log-domain Sinkhorn equals, in the probability
domain, classic Sinkhorn scaling of K = exp(masked_logits):

    v_0 = 1;  u_k = 1/(K v_{k-1} + eps);  v_k = 1/(K^T u_k + eps)   (k = 1..5)
    out = K * (u_5 outer v_5) * exp(1e-6),   masked entries exactly 0.

V2: fp16 HBM I/O (halves DMA, the roofline term), float32r compute on-chip.
K^T is built with PE transposes instead of loading a host-transposed copy
(v1 spent 16 MiB/core of HBM traffic on that). The eps guard rides on the PE
as a rank-1 PSUM accumulate, so no DVE max pass. Final out = K*(u x v):
PE broadcasts v-rows per sample into PSUM, ACT/DVE apply the per-partition
u scale (w = ps_b * u, written into the dead et tile), DVE/Pool multiply.

Emission is software-pipelined over cohort PAIRS so the per-engine in-order
streams interleave across cohorts: while pair p iterates (PE/DVE ping-pong),
ACT runs pair p+1's exp and the final-phase scale-copies of pair p-1, Pool
multiplies pair p-1, DMA streams pair p+1 in / p-1 out. Within a pair the
ten matvec-reciprocal phases alternate cohorts A/B so each engine's
dependency stalls are filled by the sibling cohort.

Host-side prep (cheap numpy): mask logits to -1e4, cast fp16; output is cast
back to fp32. All O(B*A*T) math runs on device.
"""

import numpy as np

B, A, T = 512, 256, 256
NCORES = 8
BPC = B // NCORES          # samples per core
C = 8                      # cohort size (samples in lockstep)
G = BPC // C               # cohorts per core
P = G // 2                 # cohort pairs (pipeline unit)
ITERS = 5
MASKVAL = np.float16(-1e4)  # exp(-1e4) == 0.0 exactly
EPS = 1e-18                 # rank-1 PSUM bias; guards 1/0 on masked rows/cols.
                            # Added (not max'ed) into every row/col sum, so it must
                            # sit far below the smallest valid sum (~1e-13) while
                            # keeping 1/EPS^2 = 1e36 finite in fp32 for fully-masked
                            # row x column pairs in the u*v broadcast.
OUT_SCALE = float(np.exp(np.float64(1e-6)))  # reference's exp(x + 1e-6)

# --- engine-assignment knobs (per cohort) ---
ET_ACT = 7     # of the 8 et-drain chunks per cohort, how many go to ACT
W_ACT = 5      # of the 16 w scale-copies per cohort, how many go to ACT
MUL_POOL = 3   # of the 4 [128,1024] mul chunks per cohort on Pool (rest DVE)
TR_WEAVE_FROM = 4  # first iteration phase that carries transpose units
UNIT_SPREAD = 4    # FN sample-groups over which leftover TR units spread
PH_PER_GROUP = 1   # next-pair IT phases woven per FN sample-group

_NC_CACHE = None


def _build_nc():
    import concourse.tile as tile
    from concourse import bacc, mybir

    f32 = mybir.dt.float32
    f32r = mybir.dt.float32r
    fp16 = mybir.dt.float16
    AF = mybir.ActivationFunctionType

    nc = bacc.Bacc()
    lg = nc.dram_tensor("lg", [BPC, A, T], fp16, kind="ExternalInput")
    ident = nc.dram_tensor("ident", [128, 128], f32r, kind="ExternalInput")
    identsc = nc.dram_tensor("identsc", [128, 128], f32r, kind="ExternalInput")
    # sel[k, b*128+m] = 1 if k == b else 0 (v-row -> per-sample broadcast)
    sel = nc.dram_tensor("sel", [C, C * 128], f32r, kind="ExternalInput")
    # consts[0, 0:128] = EPS; consts[0, 128:128+8C] = 1.0
    consts = nc.dram_tensor("consts", [1, 256], f32r, kind="ExternalInput")
    onesd = nc.dram_tensor("onesd", [128, 8 * C], f32r, kind="ExternalInput")
    out = nc.dram_tensor("out", [BPC, A, T], fp16, kind="ExternalOutput")

    SLAB = C * 512  # free elems per cohort slab (per sample: 2 halves x 256)

    with tile.TileContext(nc) as tc:
        with (
            tc.tile_pool(name="itp", bufs=6) as itp,
            tc.tile_pool(name="e0p", bufs=5) as e0p,
            tc.tile_pool(name="etp", bufs=4) as etp,
            tc.tile_pool(name="uvp", bufs=10) as uvp,
            tc.tile_pool(name="vrowp", bufs=2) as vrowp,
            tc.tile_pool(name="constp", bufs=1) as constp,
            tc.tile_pool(name="pstp", bufs=3, space="PSUM") as pstp,
            tc.tile_pool(name="psuv", bufs=2, space="PSUM") as psuv,
            tc.tile_pool(name="psb", bufs=3, space="PSUM") as psbp,
        ):
            idr = constp.tile([128, 128], f32r)
            nc.sync.dma_start(idr[:], ident[:])
            idsc = constp.tile([128, 128], f32r)
            nc.sync.dma_start(idsc[:], identsc[:])
            sel_sb = constp.tile([C, C * 128], f32r)
            nc.sync.dma_start(sel_sb[0:C, :], sel[:])
            cst = constp.tile([1, 256], f32r)
            nc.sync.dma_start(cst[:], consts[:])
            ones2 = constp.tile([128, 8 * C], f32r)
            nc.sync.dma_start(ones2[:], onesd[:])

            st = {}  # per-cohort pipeline state: tiles

            def emit_ld(g):
                it = itp.tile([128, SLAB], fp16, name="it")
                hc = C // 2
                for h in range(2):
                    src = lg[g * C + h * hc:g * C + (h + 1) * hc].rearrange(
                        "b (h p) j -> p b h j", p=128)
                    nc.sync.dma_start(
                        it[:, h * hc * 512:(h + 1) * hc * 512].rearrange(
                            "p (b h j) -> p b h j", h=2, j=256), src)
                st[g] = {"it": it}

            def emit_ex(g):
                it = st[g]["it"]
                e0r = e0p.tile([128, SLAB], f32r, name="e0r")
                for h in range(2):
                    sl = slice(h * SLAB // 2, (h + 1) * SLAB // 2)
                    nc.scalar.activation(e0r[:, sl], it[:, sl], AF.Exp)
                st[g]["e0r"] = e0r

            def emit_tr_unit(g, b):
                # et[p=j', b*512+jt*256+ia*128+i'] = K[b, ia*128+i', jt*128+j']
                e0r, et = st[g]["e0r"], st[g]["et"]
                pst = pstp.tile([128, 512], f32r, name="pst")
                for ia in range(2):
                    for jt in range(2):
                        nc.tensor.transpose(
                            pst[:, jt * 256 + ia * 128:
                                jt * 256 + ia * 128 + 128],
                            e0r[:, b * 512 + ia * 256 + jt * 128:
                                b * 512 + ia * 256 + jt * 128 + 128],
                            idr[:],
                        )
                dst = et[:, b * 512:(b + 1) * 512]
                ea = first_ea[0] if first_ea[0] is not None else ET_ACT
                if b < ea:
                    nc.scalar.activation(dst, pst[:], AF.Copy)
                else:
                    nc.vector.tensor_copy(dst, pst[:])

            def emit_tr_alloc(g):
                st[g]["et"] = etp.tile([128, SLAB], f32r, name="et")

            def _phase_u(g):
                et = st[g]["et"]
                v2t, v2o = st[g]["v2"]
                ps_u = psuv.tile([128, 4 * C], f32, name="ps_uv")
                nc.tensor.matmul(ps_u[:], lhsT=cst[:, 0:128],
                                 rhs=cst[:, 128:128 + 4 * C],
                                 start=True, stop=False, skip_group_check=True)
                for b in range(C):
                    for ia in range(2):
                        col = 2 * (ia * C + b)
                        for jt in range(2):
                            nc.tensor.matmul(
                                ps_u[:, col:col + 2],
                                lhsT=et[:, b * 512 + jt * 256 + ia * 128:
                                        b * 512 + jt * 256 + ia * 128 + 128],
                                rhs=v2t[:, v2o + 2 * (jt * C + b):
                                        v2o + 2 * (jt * C + b) + 2],
                                start=False, stop=(jt == 1),
                                skip_group_check=True,
                            )
                u2 = uvp.tile([128, 4 * C], f32r, name="u2")
                with nc.allow_low_precision(reason="f32r scale vectors"):
                    nc.vector.reciprocal(
                        u2[:].rearrange("p (c d) -> p c d", d=2),
                        ps_u[:].rearrange("p (c d) -> p c d", d=2)
                        [:, :, 0:1].broadcast_to([128, 2 * C, 2]))
                st[g]["u2"] = (u2, 0)

            def _phase_v(g):
                e0r = st[g]["e0r"]
                u2t, u2o = st[g]["u2"]
                ps_v = psuv.tile([128, 4 * C], f32, name="ps_uv")
                nc.tensor.matmul(ps_v[:], lhsT=cst[:, 0:128],
                                 rhs=cst[:, 128:128 + 4 * C],
                                 start=True, stop=False, skip_group_check=True)
                for b in range(C):
                    for jt in range(2):
                        col = 2 * (jt * C + b)
                        for ia in range(2):
                            nc.tensor.matmul(
                                ps_v[:, col:col + 2],
                                lhsT=e0r[:, b * 512 + ia * 256 + jt * 128:
                                         b * 512 + ia * 256 + jt * 128 + 128],
                                rhs=u2t[:, u2o + 2 * (ia * C + b):
                                        u2o + 2 * (ia * C + b) + 2],
                                start=False, stop=(ia == 1),
                                skip_group_check=True,
                            )
                v2 = uvp.tile([128, 4 * C], f32r, name="v2")
                with nc.allow_low_precision(reason="f32r scale vectors"):
                    nc.vector.reciprocal(
                        v2[:].rearrange("p (c d) -> p c d", d=2),
                        ps_v[:].rearrange("p (c d) -> p c d", d=2)
                        [:, :, 0:1].broadcast_to([128, 2 * C, 2]))
                st[g]["v2"] = (v2, 0)

            def it_pair_gen(a, b):
                # yields once per phase-half (10 yields for 5 iterations);
                # the scheduler weaves other work between phase-halves
                for g in (a, b):
                    st[g]["v2"] = (ones2, 0)
                for itn in range(ITERS):
                    for half in range(2):
                        for g in (a, b):
                            if half == 0:
                                _phase_u(g)
                            else:
                                _phase_v(g)
                        yield

            def _fn_pre(g):
                u2t, u2o = st[g]["u2"]
                v2t, v2o = st[g]["v2"]
                u32 = uvp.tile([128, 2 * C], f32, name="u32")
                nc.vector.tensor_copy(u32[:], u2t[:, u2o:u2o + 4 * C:2])
                ps_vr = psbp.tile([C, 256], f32r, name="ps_b")
                for jt in range(2):
                    nc.tensor.transpose(
                        ps_vr[0:C, jt * 128:(jt + 1) * 128],
                        v2t[:, v2o + 2 * jt * C:v2o + 2 * (jt + 1) * C:2],
                        idsc[:],
                    )
                vrow = vrowp.tile([C, 256], f32r, name="vrow")
                nc.vector.tensor_copy(vrow[0:C, :], ps_vr[0:C, :])
                st[g]["u32"], st[g]["vrow"] = u32, vrow

            def _fn_bcast2(g, b2):
                # one psum bank holds two samples' v-row broadcasts
                vrow = st[g]["vrow"]
                ps_b = psbp.tile([128, 512], f32, name="ps_b")
                for o in range(2):
                    b = 2 * b2 + o
                    nc.tensor.matmul(
                        ps_b[:, o * 256:(o + 1) * 256],
                        lhsT=sel_sb[0:C, b * 128:(b + 1) * 128],
                        rhs=vrow[0:C, :], start=True, stop=True,
                    )
                st[g].setdefault("ps_b", {})[b2] = ps_b

            def _fn_sample(g, b, wi):
                # w = ps_b * u for sample b, written into the dead et tile
                et, u32 = st[g]["et"], st[g]["u32"]
                ps_b = st[g]["ps_b"][b // 2]
                o = (b % 2) * 256
                for ia in range(2):
                    wsl = slice(b * 512 + ia * 256, b * 512 + (ia + 1) * 256)
                    wa = last_wa[0] if last_wa[0] is not None else W_ACT
                    if (2 * b + ia) * wa // 16 != (2 * b + ia - 1) * wa // 16:
                        nc.scalar.activation(
                            et[:, wsl], ps_b[:, o:o + 256], AF.Copy,
                            scale=u32[:, ia * C + b:ia * C + b + 1])
                    else:
                        nc.vector.tensor_scalar_mul(
                            et[:, wsl], ps_b[:, o:o + 256],
                            u32[:, ia * C + b:ia * C + b + 1])

            def _fn_mul(g, ch):
                # out halves: [128,1024] chunks -> fp16 into the it tile.
                # The drain-tail pair splits muls evenly so idle DVE helps.
                it, e0r, et = st[g]["it"], st[g]["e0r"], st[g]["et"]
                msl = slice(ch * 1024, (ch + 1) * 1024)
                mp = last_mp[0] if last_mp[0] is not None else MUL_POOL
                if ch % (C // 2) < mp:
                    nc.gpsimd.tensor_mul(it[:, msl], e0r[:, msl], et[:, msl])
                else:
                    nc.vector.tensor_mul(it[:, msl], e0r[:, msl], et[:, msl])

            def _st_half(g, h):
                it = st[g]["it"]
                hc = C // 2
                dst = out[g * C + h * hc:g * C + (h + 1) * hc].rearrange(
                    "b (h p) j -> p b h j", p=128)
                nc.sync.dma_start(
                    dst, it[:, h * hc * 512:(h + 1) * hc * 512].rearrange(
                        "p (b h j) -> p b h j", h=2, j=256))

            def emit_fn_pair(pa, pb, tr_units=(), next_gen=None):
                # interleave the two cohorts' final phases; weave the next
                # pair's remaining transpose units between early sample
                # groups, then start the NEXT pair's iteration phases
                _fn_pre(pa)
                _fn_pre(pb)
                wi = {pa: 0, pb: 0}
                done = {pa: 0, pb: 0}
                nu = len(tr_units)
                ti = 0
                for b in range(C):
                    if b % 2 == 0:
                        for g in (pa, pb):
                            _fn_bcast2(g, b // 2)
                    for g in (pa, pb):
                        _fn_sample(g, b, wi[g])
                        wi[g] += 2
                    want = min(nu, (b + 1) * nu // UNIT_SPREAD)
                    while ti < want:
                        emit_tr_unit(*tr_units[ti])
                        ti += 1
                    if ti == nu and next_gen is not None:
                        for _ in range(PH_PER_GROUP):
                            next(next_gen, None)
                    # a [128,1024] mul covers 2 samples' w
                    if b % 2 == 1:
                        ch = b // 2
                        for g in (pa, pb):
                            _fn_mul(g, ch)

            pending_st = []
            last_mp = [None]
            last_wa = [None]
            first_ea = [4]
            gens = {}
            for s in range(P + 1):
                for g in pending_st:
                    _st_half(g, 0)
                    _st_half(g, 1)
                pending_st = []
                if s < P:
                    a, b = 2 * s, 2 * s + 1
                    emit_ld(a)
                    emit_ld(b)
                    emit_ex(a)
                    emit_ex(b)
                    emit_tr_alloc(a)
                    emit_tr_alloc(b)
                    units = [(g, sb) for sb in range(C) for g in (a, b)]
                    gens[s] = it_pair_gen(a, b)
                else:
                    units = []
                first_ea[0] = 4 if s == 0 else None
                if s >= 1:
                    p = s - 1
                    if s == P:
                        last_mp[0] = 2
                        last_wa[0] = 8
                    pa, pb = 2 * p, 2 * p + 1
                    # finish IT(p): remaining phases woven with first half
                    # of TR(s) units
                    gen = gens[p]
                    nw = len(units) // 2
                    ti = 0
                    ph = 0
                    while True:
                        if next(gen, "END") == "END":
                            break
                        ph += 1
                        if ph > TR_WEAVE_FROM:
                            want = min(nw, (ph - TR_WEAVE_FROM) * nw // 4)
                            while ti < want:
                                emit_tr_unit(*units[ti])
                                ti += 1
                    emit_fn_pair(pa, pb, units[ti:], gens.get(s))
                    pending_st = [pa, pb]
                else:
                    for u in units:
                        emit_tr_unit(*u)

            for g in pending_st:
                _st_half(g, 0)
                _st_half(g, 1)

    nc.compile()
    return nc


def _get_nc():
    global _NC_CACHE
    if _NC_CACHE is None:
        _NC_CACHE = _build_nc()
    return _NC_CACHE


def _prep_in_maps(logits, free_agents_num, tasks_num):
    logits = np.asarray(logits, dtype=np.float32)
    free = np.asarray(free_agents_num).astype(np.int64)
    tasks = np.asarray(tasks_num).astype(np.int64)
    row_ok = np.arange(A, dtype=np.int64)[None, :] < free[:, None]   # [B, A]
    col_ok = np.arange(T, dtype=np.int64)[None, :] < tasks[:, None]  # [B, T]
    mask = row_ok[:, :, None] & col_ok[:, None, :]
    lgm = np.where(mask, logits, np.float32(MASKVAL)).astype(np.float16)
    ident = np.eye(128, dtype=np.float32)
    identsc = (ident * OUT_SCALE).astype(np.float32)
    sel = np.zeros((C, C * 128), dtype=np.float32)
    for b in range(C):
        sel[b, b * 128:(b + 1) * 128] = 1.0
    consts = np.zeros((1, 256), dtype=np.float32)
    consts[0, 0:128] = EPS
    consts[0, 128:128 + 8 * C] = 1.0
    return [
        {
            "lg": np.ascontiguousarray(lgm[c * BPC:(c + 1) * BPC]),
            "ident": ident,
            "identsc": identsc,
            "sel": sel,
            "consts": consts,
            "onesd": np.ones((128, 8 * C), dtype=np.float32),
        }
        for c in range(NCORES)
    ]


def _run(logits, free_agents_num, tasks_num, **spmd_kwargs):
    from concourse.bass_utils import run_bass_kernel_spmd

    in_maps = _prep_in_maps(logits, free_agents_num, tasks_num)
    res = run_bass_kernel_spmd(
        _get_nc(), in_maps, core_ids=list(range(NCORES)), **spmd_kwargs
    )
    out = np.concatenate(
        [r["out"].astype(np.float32) for r in res.results], axis=0)
    return out, res


def kernel(logits, free_agents_num, tasks_num):
    out, _ = _run(logits, free_agents_num, tasks_num)
    return out
